# revision 1
# baseline (speedup 1.0000x reference)
"""TRN2 Bass kernel for nn_KnnModule (retrieval_knn).

Strategy (sharded over the 8 NeuronCores):
  - train set (100000 x 1024) is padded to 102400 rows and split into 8
    chunks of 12800; each core computes sims = features @ chunk.T
    (2048 x 12800) on the PE (float32r single-pass, 1 cycle/row), and for
    every 512-wide tile of the chunk extracts the top-8 values + indices
    per row with the DVE InstMax/InstMaxIndex top-8 primitives
    (25 tiles -> 200 candidates per row per core).
  - host merges the 8 x 200 per-row candidate lists, takes the top-48 by
    approximate value, recomputes their sims exactly in fp32 (48 dot
    products per row ~ 0.03% of the device FLOPs), and reproduces the
    reference softmax voting for k in (10, 20, 100, 200).

  Why this is exact: with T=0.07 and sims ~ N(0, 37^2), a candidate's
  fp32 softmax weight is exactly 0.0 unless its sim is within ~7.3 of the
  row max. On this distribution there are at most ~15 such candidates per
  row and at most 2 per 512-wide tile (4x safety margin vs the top-8
  extraction), and float32r's max matmul error (1.8e-2) is negligible vs
  the 7.3 window. Rows that get anywhere near these margins are detected
  and recomputed exactly on the host; on this data the triggers never
  fire.
"""

import numpy as np

KS = (10, 20, 100, 200)
T = 0.07
NUM_CLASSES = 1000
B, N, D = 2048, 100000, 1024
NCORES = 8
NCHUNK = 12800  # per-core padded chunk (12500 real + 300 zero pad)
TILE_N = 512
NT = NCHUNK // TILE_N  # 25 tiles -> 200 candidate slots per core
P = 128
KEXACT = 48  # candidates per row exactly rescored on host

_NC_CACHE = {}


def _build_bass():
    import concourse.bacc as bacc
    import concourse.mybir as mybir
    import concourse.tile as tile

    mm_dtype = mybir.dt.float32r
    KO = D // P
    MB = B // P

    nc = bacc.Bacc(
        "TRN2",
        target_bir_lowering=False,
        debug=False,
        enable_asserts=False,
    )
    featT = nc.dram_tensor("featT", (D, B), mm_dtype, kind="ExternalInput")
    trainT = nc.dram_tensor("trainT", (D, NCHUNK), mm_dtype, kind="ExternalInput")
    out_val = nc.dram_tensor("t8val", (B, NT * 8), mybir.dt.float32, kind="ExternalOutput")
    out_idx = nc.dram_tensor("t8idx", (B, NT * 8), mybir.dt.uint16, kind="ExternalOutput")

    featT_ap = featT.ap().rearrange("(ko p) b -> p ko b", p=P)
    trainT_ap = trainT.ap().rearrange("(ko p) n -> p ko n", p=P)

    with tile.TileContext(nc) as tc:
        with (
            tc.tile_pool(name="const", bufs=1) as cpool,
            tc.tile_pool(name="stream", bufs=2) as spool,
            tc.tile_pool(name="acc", bufs=1) as apool,
            tc.tile_pool(name="psum", bufs=8, space="PSUM") as ppool,
        ):
            feat_sb = cpool.tile([P, KO, B], mm_dtype)
            # single DMA measured fastest: startup is HBM-BW-bound (10MB of
            # feat+train before ~30us of matmul work), so splitting this
            # into per-m chunks only delays the train tile behind 16 queued
            # transfers (measured +9us)
            nc.sync.dma_start(feat_sb, featT_ap)

            val_sb = [
                apool.tile([P, NT * 8], mybir.dt.float32, name=f"val_sb_{m}", tag=f"val{m}")
                for m in range(MB)
            ]
            idx_sb = [
                apool.tile([P, NT * 8], mybir.dt.uint16, name=f"idx_sb_{m}", tag=f"idx{m}")
                for m in range(MB)
            ]

            # train tiles processed in pairs: the same feat[ko,m] weights
            # feed two rhs tiles back-to-back, halving LDWEIGHTS pressure
            # (f32r gets no fast-weight-load; LDW=187ns vs 213ns matmul)
            t = 0
            while t < NT:
                G = 2 if t + 1 < NT else 1
                tr_sb = spool.tile(
                    [P, KO, 2 * TILE_N], mm_dtype, name="tr_sb", tag="train"
                )
                nc.sync.dma_start(
                    tr_sb[:, :, : G * TILE_N],
                    trainT_ap[:, :, t * TILE_N : (t + G) * TILE_N],
                )
                for m in range(MB):
                    pss = [
                        ppool.tile([P, TILE_N], mybir.dt.float32, name="ps", tag="ps")
                        for _ in range(G)
                    ]
                    for ko in range(KO):
                        for g in range(G):
                            nc.tensor.matmul(
                                pss[g],
                                lhsT=feat_sb[:, ko, m * P : (m + 1) * P],
                                rhs=tr_sb[:, ko, g * TILE_N : (g + 1) * TILE_N],
                                start=(ko == 0),
                                stop=(ko == KO - 1),
                            )
                    for g in range(G):
                        tt = t + g
                        vslice = val_sb[m][:, tt * 8 : (tt + 1) * 8]
                        nc.vector.max(out=vslice, in_=pss[g])
                        nc.vector.max_index(
                            out=idx_sb[m][:, tt * 8 : (tt + 1) * 8],
                            in_max=vslice,
                            in_values=pss[g],
                        )
                t += G

            ov = out_val.ap().rearrange("(mb p) c -> mb p c", p=P)
            oi = out_idx.ap().rearrange("(mb p) c -> mb p c", p=P)
            for m in range(MB):
                nc.sync.dma_start(ov[m], val_sb[m])
                nc.sync.dma_start(oi[m], idx_sb[m])

    nc.compile()
    return nc


def _get_nc():
    if "nc" not in _NC_CACHE:
        _NC_CACHE["nc"] = _build_bass()
    return _NC_CACHE["nc"]


def _vote(topv, labels):
    """Reproduce the reference's softmax voting given sorted top sims.

    topv: (B', 200) fp32 descending (padded with -inf); labels (B', 200).
    """
    Bp = topv.shape[0]
    x = (topv / np.float32(T)).astype(np.float32)
    e = np.exp(x - x[:, :1], dtype=np.float32)
    s = e.sum(axis=1, keepdims=True, dtype=np.float32)
    w = (e / s).astype(np.float32)
    rows = np.broadcast_to(np.arange(Bp)[:, None], labels.shape)
    outs = []
    for k in KS:
        p = np.zeros((Bp, NUM_CLASSES), np.float32)
        np.add.at(p, (rows[:, :k], labels[:, :k]), w[:, :k])
        outs.append(p)
    return outs


def _exact_row(F, TR, LB, b):
    s = (F[b : b + 1] @ TR.T).astype(np.float32)[0]
    o = np.argsort(-s, kind="stable")[:200]
    return _vote(s[o][None].astype(np.float32), LB[o].astype(np.int64)[None])


def _combine(F, TR, LB, vals, idxs):
    NTN = NT * 8
    slot_tile = (np.arange(NTN) // 8) * TILE_N
    gcol = (
        idxs
        + slot_tile[None, None, :]
        + (np.arange(NCORES)[:, None, None] * NCHUNK)
    )
    v = vals.transpose(1, 0, 2).reshape(B, NCORES * NTN)
    g = gcol.transpose(1, 0, 2).reshape(B, NCORES * NTN)
    v = np.where(g < N, v, -np.inf).astype(np.float32)

    # approximate top-KEXACT per row
    part = np.argpartition(-v, KEXACT, axis=1)[:, :KEXACT]
    rows = np.arange(B)[:, None]
    cand_v = v[rows, part]
    cand_g = g[rows, part]

    # exact fp32 rescoring of the candidates (0.03% of device FLOPs)
    exact = np.einsum(
        "bkd,bd->bk", TR[cand_g], F, optimize=True
    ).astype(np.float32)

    # sort by exact value desc, ties by train index asc (lax.top_k order)
    ordk = np.lexsort((cand_g, -exact.astype(np.float64)), axis=1)
    exact_s = np.take_along_axis(exact, ordk, axis=1)
    g_s = np.take_along_axis(cand_g, ordk, axis=1)

    topv = np.full((B, 200), -np.inf, np.float32)
    topv[:, :KEXACT] = exact_s
    labels = np.zeros((B, 200), np.int64)
    labels[:, :KEXACT] = LB[g_s].astype(np.int64)

    outs = _vote(topv, labels)

    # pathological-row triggers -> exact host recompute
    amax = cand_v.max(axis=1)
    # (i) too many candidates near the top (exact-significance window overflow)
    near = (cand_v >= (amax[:, None] - 8.0)).sum(axis=1)
    trig_i = near >= KEXACT - 8
    # (ii) some tile's 8th approx value near the top (dropped 9th candidate)
    v8 = vals[:, :, 7::8]  # (ncores, B, NT)
    trig_ii = v8.max(axis=(0, 2)) >= amax - 8.5
    # (iii) duplicate global col among candidates (HW tie semantics)
    ss = np.sort(cand_g, axis=1)
    trig_iii = (np.diff(ss, axis=1) == 0).any(axis=1)

    for b in np.where(trig_i | trig_ii | trig_iii)[0]:
        ob = _exact_row(F, TR, LB, b)
        for i in range(len(KS)):
            outs[i][b] = ob[i][0]

    return tuple(outs)


def kernel(features_rank, train_features, train_labels):
    from concourse.bass_utils import run_bass_kernel_spmd

    F = np.ascontiguousarray(np.asarray(features_rank, dtype=np.float32))
    TR = np.ascontiguousarray(np.asarray(train_features, dtype=np.float32))
    LB = np.asarray(train_labels)

    TRp = np.zeros((NCORES * NCHUNK, D), np.float32)
    TRp[:N] = TR
    featT = np.ascontiguousarray(F.T)

    in_maps = [
        {
            "featT": featT,
            "trainT": np.ascontiguousarray(TRp[c * NCHUNK : (c + 1) * NCHUNK].T),
        }
        for c in range(NCORES)
    ]

    nc = _get_nc()
    res = run_bass_kernel_spmd(nc, in_maps, core_ids=list(range(NCORES)))

    vals = np.stack([np.asarray(res.results[c]["t8val"]) for c in range(NCORES)])
    idxs = np.stack(
        [np.asarray(res.results[c]["t8idx"]).astype(np.int64) for c in range(NCORES)]
    )
    return _combine(F, TR, LB, vals, idxs)



# revision 2
# speedup vs baseline: 2.0373x; 2.0373x over previous
"""TRN2 Bass kernel for nn_KnnModule (retrieval_knn).

Strategy (sharded over the 8 NeuronCores):
  - train set (100000 x 1024) is padded to 102400 rows and split into 8
    chunks of 12800; each core computes sims = features @ chunk.T
    (2048 x 12800) on the PE in fp8e4m3 with DoubleRow double-pumping
    (2 fp8 MACs/cell/cycle -> ~2x the fp32r/bf16 matmul rate).
  - the (128, 512) fp32 PSUM sim tiles are staged to SBUF as fp16 by the
    Act engine; the DVE folds each pair of tiles (1024 cols) with a
    3-level tensor_tensor max tree into 128 group-maxima (groups of 8
    columns, stride 128), accumulating a per-row vector of 1664 fp16
    group maxima (12 tile pairs + the odd 25th tile as 128 groups of 4).
  - the DVE extracts the top-8 (value, index) group candidates per row
    from each of two halves of that vector (InstMax/InstMaxIndex), so
    every row leaves the device as 16 (fp16 value, uint16 group) pairs
    per core.
  - host merges the 8x16 group candidates per row, exactly rescores the
    members of the top NG=32 groups (256 candidate columns) in fp32, and
    reproduces the reference softmax voting for k in (10, 20, 100, 200).

  Why this is exact: with T=0.07 softmax weights are exactly 0.0 in fp32
  unless the sim is within ~7.4 of the row max.  Group maxima upper-bound
  their members, fp8 sim error is <~7 absolute (measured 5sigma ~ 6.2)
  and fp16 group quantization <~0.13, so every needed column lives in a
  group whose fp16 value ranks <= ~11 globally (measured <= 11 over 512
  rows); NG=32 rescored groups leaves 3x margin.  Rows where the margins
  could be violated (12th group within 16 of the max, a duplicated
  group index near the top from an fp16 tie, or a part's 8th slot near
  the top) are detected and recomputed exactly on the host (~1-2% of
  rows, batched into one GEMM).
"""

import numpy as np
import ml_dtypes

KS = (10, 20, 100, 200)
T = 0.07
NUM_CLASSES = 1000
B, N, D = 2048, 100000, 1024
NCORES = 8
NCHUNK = 12800  # per-core padded chunk (12500 real + 300 zero pad)
TILE_N = 512
NPAIR = 12           # tile pairs folded into groups of 8
NGROUP = 13 * 128    # 1664 groups per (row, core-chunk)
PARTA = 1024         # gmax cols [0, 1024) = odd tile + pairs 0..6
PARTB = NGROUP - PARTA
P = 128
MB = B // P
KO = D // P
NG = 32              # groups exactly rescored per row
MARGIN = 16.0        # host fallback margin (window 7.4 + fp8 + fp16 err)

F8_DT = ml_dtypes.float8_e4m3

_NC_CACHE = {}


def _build_bass():
    import concourse.bacc as bacc
    import concourse.mybir as mybir
    import concourse.tile as tile

    f8 = mybir.dt.float8e4
    f16 = mybir.dt.float16
    f32 = mybir.dt.float32
    u16 = mybir.dt.uint16
    DR = mybir.MatmulPerfMode.DoubleRow
    MAX = mybir.AluOpType.max

    nc = bacc.Bacc(
        "TRN2",
        target_bir_lowering=False,
        debug=False,
        enable_asserts=False,
    )
    featT = nc.dram_tensor("featT", (D, B), f8, kind="ExternalInput")
    trainT = nc.dram_tensor("trainT", (D, NCHUNK), f8, kind="ExternalInput")
    out_val = nc.dram_tensor("gval", (B, 16), f16, kind="ExternalOutput")
    out_idx = nc.dram_tensor("gidx", (B, 16), u16, kind="ExternalOutput")

    featT_ap = featT.ap().rearrange("(ko p) b -> p ko b", p=P)
    trainT_ap = trainT.ap().rearrange("(ko p) n -> p ko n", p=P)

    with tile.TileContext(nc) as tc:
        with (
            tc.tile_pool(name="const", bufs=1) as cpool,
            tc.tile_pool(name="stream", bufs=2) as spool,
            tc.tile_pool(name="acc", bufs=1) as apool,
            tc.tile_pool(name="stage", bufs=4) as xpool,
            tc.tile_pool(name="fold", bufs=2) as fpool,
            tc.tile_pool(name="psum", bufs=8, space="PSUM") as ppool,
        ):
            feat_sb = cpool.tile([P, KO, B], f8)
            nc.sync.dma_start(feat_sb, featT_ap)

            gmax_sb = [
                apool.tile([P, NGROUP], f16, name=f"gmax_{m}", tag=f"gm{m}")
                for m in range(MB)
            ]
            val_sb = [
                apool.tile([P, 16], f16, name=f"val_{m}", tag=f"v{m}")
                for m in range(MB)
            ]
            idx_sb = [
                apool.tile([P, 16], u16, name=f"idx_{m}", tag=f"i{m}")
                for m in range(MB)
            ]

            def mm_tile(ps, tr_sb, m, col):
                # one (128, 512) sim tile: 4 DoubleRow matmuls over ko pairs
                for kop in range(KO // 2):
                    nc.tensor.matmul(
                        ps,
                        lhsT=feat_sb[:, 2 * kop : 2 * kop + 2, m * P : (m + 1) * P],
                        rhs=tr_sb[:, 2 * kop : 2 * kop + 2, col : col + TILE_N],
                        start=(kop == 0),
                        stop=(kop == KO // 2 - 1),
                        perf_mode=DR,
                    )

            def top8(m, lo, hi, slot):
                vs = val_sb[m][:, slot : slot + 8]
                nc.vector.max(out=vs, in_=gmax_sb[m][:, lo:hi])
                nc.vector.max_index(
                    out=idx_sb[m][:, slot : slot + 8],
                    in_max=vs,
                    in_values=gmax_sb[m][:, lo:hi],
                )

            # odd tile first (chunk cols 12288:12800 -> gmax[:, 0:128],
            # 128 groups of 4: cols 12288 + j + 128k, k<4)
            tr_sb = spool.tile([P, KO, 2 * TILE_N], f8, name="tr_sb", tag="train")
            nc.sync.dma_start(
                tr_sb[:, :, :TILE_N], trainT_ap[:, :, NPAIR * 1024 : NCHUNK]
            )
            for m in range(MB):
                ps = ppool.tile([P, TILE_N], f32, name="ps", tag="ps")
                mm_tile(ps, tr_sb, m, 0)
                s16 = xpool.tile([P, 2 * TILE_N], f16, name="s16", tag="s16")
                nc.scalar.copy(out=s16[:, :TILE_N], in_=ps)
                fa = fpool.tile([P, 256], f16, name="fa", tag="fa")
                nc.vector.tensor_tensor(fa, s16[:, :256], s16[:, 256:512], MAX)
                nc.vector.tensor_tensor(
                    gmax_sb[m][:, 0:P], fa[:, :128], fa[:, 128:], MAX
                )

            # 12 tile pairs (chunk cols 1024p : 1024p+1024 ->
            # gmax[:, 128+128p : 256+128p], groups of 8: 1024p + j + 128k)
            for p in range(NPAIR):
                tr_sb = spool.tile([P, KO, 2 * TILE_N], f8, name="tr_sb", tag="train")
                nc.sync.dma_start(
                    tr_sb, trainT_ap[:, :, p * 1024 : (p + 1) * 1024]
                )
                for m in range(MB):
                    psA = ppool.tile([P, TILE_N], f32, name="ps", tag="ps")
                    psB = ppool.tile([P, TILE_N], f32, name="ps", tag="ps")
                    for kop in range(KO // 2):
                        for ps, col in ((psA, 0), (psB, TILE_N)):
                            nc.tensor.matmul(
                                ps,
                                lhsT=feat_sb[
                                    :, 2 * kop : 2 * kop + 2, m * P : (m + 1) * P
                                ],
                                rhs=tr_sb[:, 2 * kop : 2 * kop + 2, col : col + TILE_N],
                                start=(kop == 0),
                                stop=(kop == KO // 2 - 1),
                                perf_mode=DR,
                            )
                    s16 = xpool.tile([P, 2 * TILE_N], f16, name="s16", tag="s16")
                    nc.scalar.copy(out=s16[:, :TILE_N], in_=psA)
                    nc.scalar.copy(out=s16[:, TILE_N:], in_=psB)
                    fa = fpool.tile([P, TILE_N], f16, name="fh", tag="fh")
                    nc.vector.tensor_tensor(fa, s16[:, :TILE_N], s16[:, TILE_N:], MAX)
                    fb = fpool.tile([P, 256], f16, name="fa", tag="fa")
                    nc.vector.tensor_tensor(fb, fa[:, :256], fa[:, 256:], MAX)
                    off = P + p * P
                    nc.vector.tensor_tensor(
                        gmax_sb[m][:, off : off + P], fb[:, :128], fb[:, 128:], MAX
                    )
                    # spread the part-A top8 over pairs 7..10, part-B at 11
                    if 7 <= p <= 10 and 4 * (p - 7) <= m < 4 * (p - 7) + 4:
                        top8(m, 0, PARTA, 0)
                    if p == NPAIR - 1:
                        top8(m, PARTA, NGROUP, 8)

            ov = out_val.ap().rearrange("(mb p) c -> mb p c", p=P)
            oi = out_idx.ap().rearrange("(mb p) c -> mb p c", p=P)
            for m in range(MB):
                nc.sync.dma_start(ov[m], val_sb[m])
                nc.sync.dma_start(oi[m], idx_sb[m])

    nc.compile()
    return nc


def _get_nc():
    if "nc" not in _NC_CACHE:
        _NC_CACHE["nc"] = _build_bass()
    return _NC_CACHE["nc"]


def _group_cols():
    """(NGROUP, 8) chunk-column members per group; -1 = unused slot."""
    cols = np.full((NGROUP, 8), -1, np.int64)
    j = np.arange(128)
    k = np.arange(8)
    for p in range(NPAIR):
        cols[P + p * P : P + (p + 1) * P] = 1024 * p + j[:, None] + 128 * k[None, :]
    cols[0:P, :4] = 12288 + j[:, None] + 128 * np.arange(4)[None, :]
    return cols


_COLS = _group_cols()


def _vote(topv, labels):
    """Reproduce the reference's softmax voting given sorted top sims."""
    x = (topv / np.float32(T)).astype(np.float32)
    e = np.exp(x - x[:, :1], dtype=np.float32)
    s = e.sum(axis=1, keepdims=True, dtype=np.float32)
    w = (e / s).astype(np.float32)
    rows = np.broadcast_to(np.arange(topv.shape[0])[:, None], labels.shape)
    outs = []
    for k in KS:
        p = np.zeros((topv.shape[0], NUM_CLASSES), np.float32)
        np.add.at(p, (rows[:, :k], labels[:, :k]), w[:, :k])
        outs.append(p)
    return outs


def _exact_rows(F, TR, LB, rows, outs):
    """Batched exact recompute of the given rows (reference semantics)."""
    if len(rows) == 0:
        return
    s = (F[rows] @ TR.T).astype(np.float32)
    o = np.argsort(-s.astype(np.float64), axis=1, kind="stable")[:, :200]
    topv = np.take_along_axis(s, o, axis=1).astype(np.float32)
    labs = LB[o].astype(np.int64)
    sub = _vote(topv, labs)
    for i in range(len(KS)):
        outs[i][rows] = sub[i]


def _combine(F, TR, LB, vals, idxs):
    """vals/idxs: (NCORES, B, 16) fp32 / int64 device candidates."""
    # global group ids: slots 0..7 are part A (group = idx), 8..15 part B
    gl = idxs + np.where(np.arange(16)[None, None, :] < 8, 0, PARTA)
    gl = gl + np.arange(NCORES)[:, None, None] * NGROUP
    v = vals.transpose(1, 0, 2).reshape(B, NCORES * 16).astype(np.float32)
    g = gl.transpose(1, 0, 2).reshape(B, NCORES * 16)

    order = np.argsort(-v, axis=1)
    v_s = np.take_along_axis(v, order, axis=1)
    g_s = np.take_along_axis(g, order, axis=1)
    amax = v_s[:, 0]

    # triggers
    trig = v_s[:, NG - 1] >= amax - MARGIN  # 32nd group near window
    # per-(core,part) duplicated group index near the top (fp16 tie)
    iv = idxs.transpose(1, 0, 2).reshape(B, NCORES, 2, 8)
    vv = vals.transpose(1, 0, 2).reshape(B, NCORES, 2, 8).astype(np.float32)
    si = np.sort(iv, axis=3)
    dup = (np.diff(si, axis=3) == 0).any(axis=3) & (
        vv.max(axis=3) >= amax[:, None, None] - MARGIN
    )
    trig |= dup.any(axis=(1, 2))
    # a part's 8th slot near the top (its 9th group may be within window)
    trig |= (vv[:, :, :, 7] >= amax[:, None, None] - MARGIN).any(axis=(1, 2))

    # exact rescore of the top NG groups' member columns
    top_g = g_s[:, :NG]
    core = top_g // NGROUP
    mem = _COLS[top_g % NGROUP]  # (B, NG, 8) chunk cols, -1 pad
    gcol = mem + core[:, :, None] * NCHUNK
    valid = (mem >= 0) & (gcol < N)
    gflat = np.where(valid, gcol, 0).reshape(B, NG * 8)

    exact = np.empty((B, NG * 8), np.float32)
    step = 256
    for b0 in range(0, B, step):
        b1 = min(b0 + step, B)
        exact[b0:b1] = np.einsum(
            "bkd,bd->bk", TR[gflat[b0:b1]], F[b0:b1], optimize=True
        )
    exact = np.where(valid.reshape(B, NG * 8), exact, -np.inf)

    # sort by exact value desc, ties by train index asc (lax.top_k order)
    ordk = np.lexsort((gflat, -exact.astype(np.float64)), axis=1)
    exact_s = np.take_along_axis(exact, ordk, axis=1)[:, :200].astype(np.float32)
    col_s = np.take_along_axis(gflat, ordk, axis=1)[:, :200]

    labels = np.where(exact_s > -np.inf, LB[col_s], 0).astype(np.int64)
    outs = _vote(exact_s, labels)

    _exact_rows(F, TR, LB, np.where(trig)[0], outs)
    return tuple(outs)


def make_in_maps(F, TR):
    TRp = np.zeros((NCORES * NCHUNK, D), np.float32)
    TRp[:N] = TR
    feat8 = np.ascontiguousarray(F.astype(F8_DT).T)
    return [
        {
            "featT": feat8,
            "trainT": np.ascontiguousarray(
                TRp[c * NCHUNK : (c + 1) * NCHUNK].astype(F8_DT).T
            ),
        }
        for c in range(NCORES)
    ]


def kernel(features_rank, train_features, train_labels):
    from concourse.bass_utils import run_bass_kernel_spmd

    F = np.ascontiguousarray(np.asarray(features_rank, dtype=np.float32))
    TR = np.ascontiguousarray(np.asarray(train_features, dtype=np.float32))
    LB = np.asarray(train_labels)

    nc = _get_nc()
    res = run_bass_kernel_spmd(nc, make_in_maps(F, TR), core_ids=list(range(NCORES)))

    vals = np.stack(
        [np.asarray(res.results[c]["gval"]).astype(np.float32) for c in range(NCORES)]
    )
    idxs = np.stack(
        [np.asarray(res.results[c]["gidx"]).astype(np.int64) for c in range(NCORES)]
    )
    return _combine(F, TR, LB, vals, idxs)


# revision 10
# speedup vs baseline: 2.0670x; 1.0146x over previous
"""TRN2 Bass kernel for nn_KnnModule (retrieval_knn).

Strategy (sharded over the 8 NeuronCores):
  - train set (100000 x 1024) is padded to 102400 rows and split into 8
    chunks of 12800; each core computes sims = features @ chunk.T
    (2048 x 12800) on the PE in fp8e4m3 with DoubleRow double-pumping
    (2 fp8 MACs/cell/cycle -> ~2x the fp32r/bf16 matmul rate).
  - the (128, 512) fp32 PSUM sim tiles are staged to SBUF as fp16 by the
    Act engine; the DVE folds each pair of tiles (1024 cols) with a
    3-level tensor_tensor max tree into 128 group-maxima (groups of 8
    columns, stride 128), accumulating a per-row vector of 1664 fp16
    group maxima (12 tile pairs + the odd 25th tile as 128 groups of 4).
  - the DVE extracts the top-8 (value, index) group candidates per row
    from each of two halves of that vector (InstMax/InstMaxIndex), so
    every row leaves the device as 16 (fp16 value, uint16 group) pairs
    per core.
  - host merges the 8x16 group candidates per row, exactly rescores the
    members of the top NG=32 groups (256 candidate columns) in fp32, and
    reproduces the reference softmax voting for k in (10, 20, 100, 200).

  Why this is exact: with T=0.07 softmax weights are exactly 0.0 in fp32
  unless the sim is within ~7.4 of the row max.  Group maxima upper-bound
  their members, fp8 sim error is <~7 absolute (measured 5sigma ~ 6.2)
  and fp16 group quantization <~0.13, so every needed column lives in a
  group whose fp16 value ranks <= ~11 globally (measured <= 11 over 512
  rows); NG=32 rescored groups leaves 3x margin.  Rows where the margins
  could be violated (12th group within 16 of the max, a duplicated
  group index near the top from an fp16 tie, or a part's 8th slot near
  the top) are detected and recomputed exactly on the host (~1-2% of
  rows, batched into one GEMM).
"""

import numpy as np
import ml_dtypes

KS = (10, 20, 100, 200)
T = 0.07
NUM_CLASSES = 1000
B, N, D = 2048, 100000, 1024
NCORES = 8
NCHUNK = 12800  # per-core padded chunk (12500 real + 300 zero pad)
TILE_N = 512
NPAIR = 12           # tile pairs folded into groups of 8
NGROUP = 13 * 128    # 1664 groups per (row, core-chunk)
# gmax col parts with separate top-8 extraction, staggered to keep the
# DVE off the critical path: A = odd tile + pairs 0..6, B = pairs 7..9,
# C = pairs 10..11
PARTS = ((0, 1024), (1024, 1408), (1408, 1664))
P = 128
MB = B // P
KO = D // P
NG = 32              # groups exactly rescored per row
MARGIN = 16.0        # host fallback margin (window 7.4 + fp8 + fp16 err)

F8_DT = ml_dtypes.float8_e4m3

_NC_CACHE = {}


def _build_bass():
    import concourse.bacc as bacc
    import concourse.mybir as mybir
    import concourse.tile as tile

    f8 = mybir.dt.float8e4
    f16 = mybir.dt.float16
    f32 = mybir.dt.float32
    u16 = mybir.dt.uint16
    DR = mybir.MatmulPerfMode.DoubleRow
    MAX = mybir.AluOpType.max

    nc = bacc.Bacc(
        "TRN2",
        target_bir_lowering=False,
        debug=False,
        enable_asserts=False,
    )
    featT = nc.dram_tensor("featT", (D, B), f8, kind="ExternalInput")
    trainT = nc.dram_tensor("trainT", (D, NCHUNK), f8, kind="ExternalInput")
    out_val = nc.dram_tensor("gval", (B, 24), f16, kind="ExternalOutput")
    out_idx = nc.dram_tensor("gidx", (B, 24), u16, kind="ExternalOutput")

    featT_ap = featT.ap().rearrange("(ko p) b -> p ko b", p=P)
    trainT_ap = trainT.ap().rearrange("(ko p) n -> p ko n", p=P)

    with tile.TileContext(nc) as tc:
        with (
            tc.tile_pool(name="const", bufs=1) as cpool,
            tc.tile_pool(name="stream", bufs=2) as spool,
            tc.tile_pool(name="acc", bufs=1) as apool,
            tc.tile_pool(name="stage", bufs=4) as xpool,
            tc.tile_pool(name="fold", bufs=2) as fpool,
            tc.tile_pool(name="psum", bufs=8, space="PSUM") as ppool,
        ):
            # feat DMA in 4 b-chunks so the first m-blocks unblock early
            FCH = B // 4
            feat_sb = [
                cpool.tile([P, KO, FCH], f8, name=f"feat_{c}", tag=f"f{c}")
                for c in range(4)
            ]
            for c in range(4):
                nc.sync.dma_start(
                    feat_sb[c], featT_ap[:, :, c * FCH : (c + 1) * FCH]
                )

            def feat_slice(kop, m):
                c, r = divmod(m * P, FCH)
                return feat_sb[c][:, 2 * kop : 2 * kop + 2, r : r + P]

            gmax_sb = [
                apool.tile([P, NGROUP], f16, name=f"gmax_{m}", tag=f"gm{m}")
                for m in range(MB)
            ]
            val_sb = [
                apool.tile([P, 24], f16, name=f"val_{m}", tag=f"v{m}")
                for m in range(MB)
            ]
            idx_sb = [
                apool.tile([P, 24], u16, name=f"idx_{m}", tag=f"i{m}")
                for m in range(MB)
            ]

            def mm_tile(ps, tr_sb, m, col):
                # one (128, 512) sim tile: 4 DoubleRow matmuls over ko pairs
                for kop in range(KO // 2):
                    nc.tensor.matmul(
                        ps,
                        lhsT=feat_slice(kop, m),
                        rhs=tr_sb[:, 2 * kop : 2 * kop + 2, col : col + TILE_N],
                        start=(kop == 0),
                        stop=(kop == KO // 2 - 1),
                        perf_mode=DR,
                    )

            def top8(m, lo, hi, slot):
                vs = val_sb[m][:, slot : slot + 8]
                nc.vector.max(out=vs, in_=gmax_sb[m][:, lo:hi])
                nc.vector.max_index(
                    out=idx_sb[m][:, slot : slot + 8],
                    in_max=vs,
                    in_values=gmax_sb[m][:, lo:hi],
                )

            # odd tile first (chunk cols 12288:12800 -> gmax[:, 0:128],
            # 128 groups of 4: cols 12288 + j + 128k, k<4)
            tr_sb = spool.tile([P, KO, 2 * TILE_N], f8, name="tr_sb", tag="train")
            nc.sync.dma_start(
                tr_sb[:, :, :TILE_N], trainT_ap[:, :, NPAIR * 1024 : NCHUNK]
            )
            for m in range(MB):
                ps = ppool.tile([P, TILE_N], f32, name="ps", tag="ps")
                mm_tile(ps, tr_sb, m, 0)
                s16 = xpool.tile([P, 2 * TILE_N], f16, name="s16", tag="s16")
                nc.scalar.copy(out=s16[:, :TILE_N], in_=ps)
                fa = fpool.tile([P, 256], f16, name="fa", tag="fa")
                nc.vector.tensor_tensor(fa, s16[:, :256], s16[:, 256:512], MAX)
                nc.vector.tensor_tensor(
                    gmax_sb[m][:, 0:P], fa[:, :128], fa[:, 128:], MAX
                )

            # 12 tile pairs (chunk cols 1024p : 1024p+1024 ->
            # gmax[:, 128+128p : 256+128p], groups of 8: 1024p + j + 128k)
            for p in range(NPAIR):
                tr_sb = spool.tile([P, KO, 2 * TILE_N], f8, name="tr_sb", tag="train")
                nc.sync.dma_start(
                    tr_sb, trainT_ap[:, :, p * 1024 : (p + 1) * 1024]
                )
                for m in range(MB):
                    psA = ppool.tile([P, TILE_N], f32, name="ps", tag="ps")
                    psB = ppool.tile([P, TILE_N], f32, name="ps", tag="ps")
                    for kop in range(KO // 2):
                        for ps, col in ((psA, 0), (psB, TILE_N)):
                            nc.tensor.matmul(
                                ps,
                                lhsT=feat_slice(kop, m),
                                rhs=tr_sb[:, 2 * kop : 2 * kop + 2, col : col + TILE_N],
                                start=(kop == 0),
                                stop=(kop == KO // 2 - 1),
                                perf_mode=DR,
                            )
                    s16 = xpool.tile([P, 2 * TILE_N], f16, name="s16", tag="s16")
                    nc.scalar.copy(out=s16[:, :TILE_N], in_=psA)
                    nc.scalar.copy(out=s16[:, TILE_N:], in_=psB)
                    fa = fpool.tile([P, TILE_N], f16, name="fh", tag="fh")
                    nc.vector.tensor_tensor(fa, s16[:, :TILE_N], s16[:, TILE_N:], MAX)
                    fb = fpool.tile([P, 256], f16, name="fa", tag="fa")
                    nc.vector.tensor_tensor(fb, fa[:, :256], fa[:, 256:], MAX)
                    off = P + p * P
                    nc.vector.tensor_tensor(
                        gmax_sb[m][:, off : off + P], fb[:, :128], fb[:, 128:], MAX
                    )
                    # staggered top-8 extraction:
                    #   part A (ready after pair 6): spread over pairs 7..9
                    #   part B (ready after pair 9): spread over pairs 10..11
                    #   part C (ready after pair 11): per-m at pair 11
                    if (
                        (p == 7 and m < 6)
                        or (p == 8 and 6 <= m < 11)
                        or (p == 9 and 11 <= m)
                    ):
                        top8(m, *PARTS[0], 0)
                    if (p == 10 and m < 8) or (p == 11 and 8 <= m):
                        top8(m, *PARTS[1], 8)
                    if p == NPAIR - 1:
                        top8(m, *PARTS[2], 16)

            ov = out_val.ap().rearrange("(mb p) c -> mb p c", p=P)
            oi = out_idx.ap().rearrange("(mb p) c -> mb p c", p=P)
            for m in range(MB):
                nc.sync.dma_start(ov[m], val_sb[m])
                nc.sync.dma_start(oi[m], idx_sb[m])

    nc.compile()
    return nc


def _get_nc():
    if "nc" not in _NC_CACHE:
        _NC_CACHE["nc"] = _build_bass()
    return _NC_CACHE["nc"]


def _group_cols():
    """(NGROUP, 8) chunk-column members per group; -1 = unused slot."""
    cols = np.full((NGROUP, 8), -1, np.int64)
    j = np.arange(128)
    k = np.arange(8)
    for p in range(NPAIR):
        cols[P + p * P : P + (p + 1) * P] = 1024 * p + j[:, None] + 128 * k[None, :]
    cols[0:P, :4] = 12288 + j[:, None] + 128 * np.arange(4)[None, :]
    return cols


_COLS = _group_cols()


def _vote(topv, labels):
    """Reproduce the reference's softmax voting given sorted top sims."""
    x = (topv / np.float32(T)).astype(np.float32)
    e = np.exp(x - x[:, :1], dtype=np.float32)
    s = e.sum(axis=1, keepdims=True, dtype=np.float32)
    w = (e / s).astype(np.float32)
    rows = np.broadcast_to(np.arange(topv.shape[0])[:, None], labels.shape)
    outs = []
    for k in KS:
        p = np.zeros((topv.shape[0], NUM_CLASSES), np.float32)
        np.add.at(p, (rows[:, :k], labels[:, :k]), w[:, :k])
        outs.append(p)
    return outs


def _exact_rows(F, TR, LB, rows, outs):
    """Batched exact recompute of the given rows (reference semantics)."""
    if len(rows) == 0:
        return
    s = (F[rows] @ TR.T).astype(np.float32)
    o = np.argsort(-s.astype(np.float64), axis=1, kind="stable")[:, :200]
    topv = np.take_along_axis(s, o, axis=1).astype(np.float32)
    labs = LB[o].astype(np.int64)
    sub = _vote(topv, labs)
    for i in range(len(KS)):
        outs[i][rows] = sub[i]


def _combine(F, TR, LB, vals, idxs):
    """vals/idxs: (NCORES, B, 24) fp32 / int64 device candidates."""
    # global group ids: slots 0..7 part A, 8..15 part B, 16..23 part C
    base = np.repeat([PARTS[0][0], PARTS[1][0], PARTS[2][0]], 8)
    gl = idxs + base[None, None, :]
    gl = gl + np.arange(NCORES)[:, None, None] * NGROUP
    v = vals.transpose(1, 0, 2).reshape(B, NCORES * 24).astype(np.float32)
    g = gl.transpose(1, 0, 2).reshape(B, NCORES * 24)

    order = np.argsort(-v, axis=1)
    v_s = np.take_along_axis(v, order, axis=1)
    g_s = np.take_along_axis(g, order, axis=1)
    amax = v_s[:, 0]

    # triggers
    trig = v_s[:, NG - 1] >= amax - MARGIN  # NG-th group near window
    # per-(core,part) duplicated group index near the top (fp16 tie)
    iv = idxs.transpose(1, 0, 2).reshape(B, NCORES, 3, 8)
    vv = vals.transpose(1, 0, 2).reshape(B, NCORES, 3, 8).astype(np.float32)
    si = np.sort(iv, axis=3)
    dup = (np.diff(si, axis=3) == 0).any(axis=3) & (
        vv.max(axis=3) >= amax[:, None, None] - MARGIN
    )
    trig |= dup.any(axis=(1, 2))
    # a part's 8th slot near the top (its 9th group may be within window)
    trig |= (vv[:, :, :, 7] >= amax[:, None, None] - MARGIN).any(axis=(1, 2))

    # exact rescore of the top NG groups' member columns
    top_g = g_s[:, :NG]
    core = top_g // NGROUP
    mem = _COLS[top_g % NGROUP]  # (B, NG, 8) chunk cols, -1 pad
    gcol = mem + core[:, :, None] * NCHUNK
    valid = (mem >= 0) & (gcol < N)
    gflat = np.where(valid, gcol, 0).reshape(B, NG * 8)

    exact = np.empty((B, NG * 8), np.float32)
    step = 256
    for b0 in range(0, B, step):
        b1 = min(b0 + step, B)
        exact[b0:b1] = np.einsum(
            "bkd,bd->bk", TR[gflat[b0:b1]], F[b0:b1], optimize=True
        )
    exact = np.where(valid.reshape(B, NG * 8), exact, -np.inf)

    # sort by exact value desc, ties by train index asc (lax.top_k order)
    ordk = np.lexsort((gflat, -exact.astype(np.float64)), axis=1)
    exact_s = np.take_along_axis(exact, ordk, axis=1)[:, :200].astype(np.float32)
    col_s = np.take_along_axis(gflat, ordk, axis=1)[:, :200]

    labels = np.where(exact_s > -np.inf, LB[col_s], 0).astype(np.int64)
    outs = _vote(exact_s, labels)

    _exact_rows(F, TR, LB, np.where(trig)[0], outs)
    return tuple(outs)


def make_in_maps(F, TR):
    TRp = np.zeros((NCORES * NCHUNK, D), np.float32)
    TRp[:N] = TR
    feat8 = np.ascontiguousarray(F.astype(F8_DT).T)
    return [
        {
            "featT": feat8,
            "trainT": np.ascontiguousarray(
                TRp[c * NCHUNK : (c + 1) * NCHUNK].astype(F8_DT).T
            ),
        }
        for c in range(NCORES)
    ]


def kernel(features_rank, train_features, train_labels):
    from concourse.bass_utils import run_bass_kernel_spmd

    F = np.ascontiguousarray(np.asarray(features_rank, dtype=np.float32))
    TR = np.ascontiguousarray(np.asarray(train_features, dtype=np.float32))
    LB = np.asarray(train_labels)

    nc = _get_nc()
    res = run_bass_kernel_spmd(nc, make_in_maps(F, TR), core_ids=list(range(NCORES)))

    vals = np.stack(
        [np.asarray(res.results[c]["gval"]).astype(np.float32) for c in range(NCORES)]
    )
    idxs = np.stack(
        [np.asarray(res.results[c]["gidx"]).astype(np.int64) for c in range(NCORES)]
    )
    return _combine(F, TR, LB, vals, idxs)
